# revision 3
# baseline (speedup 1.0000x reference)
"""MiniBatchDiscrimination Trainium2 kernel (symmetric window).

reference:
    proj = x @ W.T                      # [512, 500] -> [512, 100, 5]
    l1[i,j,o] = sum_k |proj[i,o,k] - proj[j,o,k]|
    mbd[i,o]  = sum_j exp(-l1[i,j,o]) - 1
    out = concat([x, mbd], axis=1)      # [512, 1124]

v3 strategy (8 cores):
  - Symmetry: core r computes its 64 rows against a 320-col cyclic
    window (own block + next 4 blocks; the distance-4 block is computed
    from both sides into own accums only). Reciprocal sums R over
    window cols [64,256) (distance 1-3) accumulate on DVE in fp16; the
    host combines mbd[row] = own_accum + sum_d R_{r-d}[64(d-1)+i] - 1.
  - DVE rows: |d| = 2*relu(d) - d and sum_k d_k telescopes:
    l1 = 2*sum_k relu(d) - Q[o,j] + P_i[o], Q = sum_k projT
    (precomputed). DVE does 4 fused tensor_scalar(subtract, max 0) ops
    per row; PSUM gets 2*sum relu via a 2.0-selector matmul plus one
    identity matmul of -Q; P_i folds into the exp bias.
  - Act rows (balance): ScalarE computes |d| directly via
    Abs(-projT + col) into the quad; 1.0-selector matmul, no Q term,
    zero exp bias.
  - exp + j-reduce: ScalarE Exp(scale=-1, bias) reading PSUM, accum_out
    -> mbdT column, E written fp16 to SBUF for the DVE R-adds.
  - Input DMA in 8 chunks per tensor, overlapped with proj matmuls.
"""

import sys

import numpy as np

sys.path.insert(0, "/opt/trn_rl_repo")

import concourse.bacc as bacc  # noqa: E402
import concourse.mybir as mybir  # noqa: E402
import concourse.tile as tile  # noqa: E402
from concourse.bass_utils import run_bass_kernel_spmd  # noqa: E402

B, IN, O, K = 512, 1024, 100, 5
OK = O * K  # 500
NCORES = 8
BL = B // NCORES  # 64 local rows per core
WIN = 5 * BL  # 320 window cols per core
RLO, RHI = BL, 4 * BL  # reciprocal cols [64, 256)
RW = RHI - RLO  # 192
NT = 4  # proj.T partition tiles
PT = OK // NT  # 125 partitions per tile
NIN = IN // 128  # 8 contraction chunks

F32 = mybir.dt.float32
F16 = mybir.dt.float16
AF = mybir.ActivationFunctionType
ALU = mybir.AluOpType

GSZ = 7  # i-rows per PSUM group (7 banks; 1 bank holds R)
HALF_ROWS = frozenset(
    (2, 7, 11, 16, 20, 25, 29, 34, 38, 43, 47, 52, 56, 61)
)
BS = 32  # sub-block size (16 blocks of 32 over B)
EW = 288  # effective window per row (9 sub-blocks)
RRLO, RRHI = 32, 256  # R region, relative to the row's 288-window
RW2 = RRHI - RRLO  # 224


def build():
    nc = bacc.Bacc("TRN2", target_bir_lowering=False)
    xT_d = nc.dram_tensor("xT", [128, NIN * WIN], F16, kind="ExternalInput")
    wT_d = nc.dram_tensor("wT", [128, NIN * OK], F16, kind="ExternalInput")
    sel2_d = nc.dram_tensor("sel2", [PT, NT, O], F16, kind="ExternalInput")
    sel1_d = nc.dram_tensor("sel1", [PT, NT, O], F16, kind="ExternalInput")
    eye_d = nc.dram_tensor("eye", [O, O], F16, kind="ExternalInput")
    mbdT_d = nc.dram_tensor("mbdT", [O, BL], F32, kind="ExternalOutput")
    r_d = nc.dram_tensor("R", [O, RW], F16, kind="ExternalOutput")

    with tile.TileContext(nc) as tc:
        with (
            tc.tile_pool(name="pers", bufs=1) as pers,
            tc.tile_pool(name="io", bufs=1) as io,
            tc.tile_pool(name="work", bufs=16) as work,
            tc.tile_pool(name="ework", bufs=8) as ework,
            tc.tile_pool(name="ps", bufs=7, space="PSUM") as ps,
            tc.tile_pool(name="rps", bufs=1, space="PSUM") as rps_pool,
        ):
            # ---- input DMA: paired chunks, issue split across sync+scalar
            xcat = io.tile([128, NIN, WIN], F16, name="xcat", tag="xcat")
            wcat = io.tile([128, NIN, OK], F16, name="wcat", tag="wcat")
            for c2 in range(NIN // 2):
                nc.sync.dma_start(
                    out=xcat[:, 2 * c2 : 2 * c2 + 2, :],
                    in_=xT_d[:, 2 * c2 * WIN : (2 * c2 + 2) * WIN],
                )
                nc.sync.dma_start(
                    out=wcat[:, 2 * c2 : 2 * c2 + 2, :],
                    in_=wT_d[:, 2 * c2 * OK : (2 * c2 + 2) * OK],
                )
            s2_all = pers.tile([PT, NT, O], F16, name="s2a", tag="s2a")
            s1_all = pers.tile([PT, NT, O], F16, name="s1a", tag="s1a")
            eye_sb = pers.tile([O, O], F16, name="eye", tag="eye")
            nc.sync.dma_start(out=s2_all[:], in_=sel2_d[:, :, :])
            nc.sync.dma_start(out=s1_all[:], in_=sel1_d[:, :, :])
            nc.sync.dma_start(out=eye_sb[:], in_=eye_d[:, :])
            s2_sb = [s2_all[:, t, :] for t in range(NT)]
            s1_sb = [s1_all[:, t, :] for t in range(NT)]

            projTb = [
                pers.tile([PT, WIN], F16, name=f"projTb{t}", tag=f"projTb{t}")
                for t in range(NT)
            ]
            projL = [
                pers.tile([PT, BL], F32, name=f"projL{t}", tag=f"projL{t}")
                for t in range(NT)
            ]
            qn_sb = pers.tile([O, WIN], F16, name="qn", tag="qn")  # -Q fp16
            pln_sb = pers.tile([O, BL], F32, name="pln", tag="pln")  # -P_i f32
            # subset (tiles 0,1) variants for half-act rows
            qnd_sb = pers.tile([O, WIN], F16, name="qnd", tag="qnd")
            plnd_sb = pers.tile([O, BL], F32, name="plnd", tag="plnd")
            mbdT_sb = pers.tile([O, BL], F32, name="mbdT_sb", tag="mbdT_sb")
            r_sb = pers.tile([O, RW], F16, name="r_sb", tag="r_sb")
            r_ps = rps_pool.tile([O, RW], F32, name="r_ps", tag="r_ps")

            # ---- proj matmuls, chunk-pipelined against the input DMA ----
            pps = [
                ps.tile([PT, WIN], F32, name=f"pps{t}", tag="ps") for t in range(NT)
            ]
            for c in range(NIN):
                for t in range(NT):
                    nc.tensor.matmul(
                        pps[t][:],
                        lhsT=wcat[:, c, PT * t : PT * (t + 1)],
                        rhs=xcat[:, c, :],
                        start=(c == 0),
                        stop=(c == NIN - 1),
                    )
            for t in range(NT):
                nc.vector.tensor_copy(projTb[t][:], pps[t][:])
                nc.scalar.copy(projL[t][:], pps[t][:, :BL])

            # ---- Q = sum_k projT over window; store -Q f16, -P f32 ----
            qps = ps.tile([O, WIN], F32, name="qps", tag="ps")
            for t in range(NT):
                nc.tensor.matmul(
                    qps[:],
                    lhsT=s2_sb[t],
                    rhs=projTb[t][:],
                    start=(t == 0),
                    stop=(t == NT - 1),
                )
            # qps = 2*Q -> qn = -Q (f16), pln = -P (f32, local cols)
            nc.vector.tensor_scalar(qn_sb[:], qps[:], -0.5, None, op0=ALU.mult)
            nc.vector.tensor_scalar(pln_sb[:], qps[:, :BL], -0.5, None, op0=ALU.mult)
            qpsd = ps.tile([O, WIN], F32, name="qpsd", tag="ps")
            for t in range(2):
                nc.tensor.matmul(
                    qpsd[:],
                    lhsT=s2_sb[t],
                    rhs=projTb[t][:],
                    start=(t == 0),
                    stop=(t == 1),
                )
            nc.vector.tensor_scalar(qnd_sb[:], qpsd[:], -0.5, None, op0=ALU.mult)
            nc.vector.tensor_scalar(
                plnd_sb[:], qpsd[:, :BL], -0.5, None, op0=ALU.mult
            )

            # ---- pairwise phase ----
            for g0 in range(0, BL, GSZ):
                gis = list(range(g0, min(g0 + GSZ, BL)))
                half = {i: i in HALF_ROWS for i in gis}
                psums = {
                    i: ps.tile([O, WIN], F32, name=f"ps{i}", tag="ps") for i in gis
                }
                aqs = {}
                for i in gis:
                    aq = work.tile([PT, NT, WIN], F16, name=f"a{i}", tag="A")
                    for t in range(NT):
                        if half[i] and t >= 2:
                            nc.scalar.activation(
                                out=aq[:, t, :],
                                in_=projTb[t][:],
                                func=AF.Abs,
                                bias=projL[t][:, i : i + 1],
                                scale=-1.0,
                            )
                        else:
                            nc.vector.tensor_scalar(
                                aq[:, t, :],
                                projTb[t][:],
                                projL[t][:, i : i + 1],
                                0.0,
                                op0=ALU.subtract,
                                op1=ALU.max,
                            )
                    aqs[i] = aq
                # w-outer matmuls: identical weights back-to-back
                for i in gis:
                    nc.tensor.matmul(
                        psums[i][:],
                        lhsT=eye_sb[:],
                        rhs=(qnd_sb if half[i] else qn_sb)[:],
                        start=True,
                        stop=False,
                    )
                for t in range(NT):
                    for i in gis:
                        sel = s1_sb[t] if (half[i] and t >= 2) else s2_sb[t]
                        nc.tensor.matmul(
                            psums[i][:],
                            lhsT=sel,
                            rhs=aqs[i][:, t, :],
                            start=False,
                            stop=(t == NT - 1),
                        )
                for i in gis:
                    e_i = ework.tile([O, WIN], F16, name=f"e{i}", tag="E")
                    nc.scalar.activation(
                        out=e_i[:],
                        in_=psums[i][:],
                        func=AF.Exp,
                        bias=(plnd_sb if half[i] else pln_sb)[:, i : i + 1],
                        scale=-1.0,
                        accum_out=mbdT_sb[:, i : i + 1],
                    )
                    # R += E[:, RLO:RHI] accumulated on the PE (PSUM bank)
                    nc.tensor.matmul(
                        r_ps[:],
                        lhsT=eye_sb[:],
                        rhs=e_i[:, RLO:RHI],
                        start=(i == 0),
                        stop=(i == BL - 1),
                        skip_group_check=True,
                    )

            nc.vector.tensor_copy(r_sb[:], r_ps[:])
            nc.sync.dma_start(out=mbdT_d[:, :], in_=mbdT_sb[:])
            nc.sync.dma_start(out=r_d[:, :], in_=r_sb[:])
    nc.compile()
    return nc


_CACHE = {}


def _build_cached():
    if "nc" not in _CACHE:
        _CACHE["nc"] = build()
    return _CACHE["nc"]


def _selector(v: float) -> np.ndarray:
    sel = np.zeros((NT, PT, O), np.float32)
    for t in range(NT):
        for p in range(PT):
            sel[t, p, (t * PT + p) % O] = v
    return sel.astype(np.float16)


def make_in_maps(x: np.ndarray, W: np.ndarray):
    xT = np.ascontiguousarray(x.T.astype(np.float16))  # [IN, B]
    # k-major proj.T rows: row p corresponds to (o = p % O, k = p // O)
    perm = np.array([(p % O) * K + p // O for p in range(OK)], np.int64)
    wTk = np.ascontiguousarray(W.T.astype(np.float16)[:, perm])  # [IN, OK]
    sel2 = np.ascontiguousarray(_selector(2.0).transpose(1, 0, 2))
    sel1 = np.ascontiguousarray(_selector(1.0).transpose(1, 0, 2))
    eye = np.eye(O, dtype=np.float16)

    def prep(a, cols):
        return np.ascontiguousarray(
            a.reshape(NIN, 128, cols).transpose(1, 0, 2).reshape(128, NIN * cols)
        )

    wprep = prep(wTk, OK)
    in_maps = []
    for r in range(NCORES):
        cols = (BL * r + np.arange(WIN)) % B
        in_maps.append(
            {
                "xT": prep(np.ascontiguousarray(xT[:, cols]), WIN),
                "wT": wprep,
                "sel2": sel2,
                "sel1": sel1,
                "eye": eye,
            }
        )
    return in_maps


def run(x, W, trace=False, **kw):
    nc = _build_cached()
    in_maps = make_in_maps(x, W)
    return run_bass_kernel_spmd(
        nc, in_maps, core_ids=list(range(NCORES)), trace=trace, **kw
    )


def kernel(x: np.ndarray, W: np.ndarray) -> np.ndarray:
    x = np.asarray(x, np.float32)
    W = np.asarray(W, np.float32)
    res = run(x, W, trace=False)
    mbd = np.empty((B, O), np.float32)
    for r in range(NCORES):
        m = res.results[r]["mbdT"].astype(np.float32).copy()  # [O, BL]
        for d in (1, 2, 3):
            c = (r - d) % NCORES
            m += res.results[c]["R"][:, BL * (d - 1) : BL * d].astype(np.float32)
        mbd[BL * r : BL * (r + 1), :] = m.T
    mbd -= 1.0
    return np.concatenate([x, mbd], axis=1)
